# revision 36
# baseline (speedup 1.0000x reference)
"""CopyGenerator kernel for 8x Trainium2 NeuronCores (Bass/Tile).

Computation (see reference):
    logits = hidden @ W.T + b            [BT, V]   (pad column masked to -inf)
    prob   = softmax(logits, axis=1)
    p_copy = sigmoid(hidden @ w_copy + b_copy)
    out    = concat([prob * (1 - p_copy),
                     einsum('bts,bsc', attn*p_copy, src_map)], axis=1)

Sharding: vocab dim of W/b/out_prob split 8 ways (tensor parallel).  Each
core computes exp(logits) for its vocab shard (bf16 matmul, tokens on
PSUM partitions), a per-token local sum-of-exp, an 8-core AllReduce of
the [BT] normalizer (tiny, pipelined in groups of 2 token tiles), and
scales+writes its out_prob columns.  The copy branch is data-parallel
over batch (2 batches per core).

All operand transposes (W -> [d-part, vocab], hidden -> [d-part, token],
attn -> [s-part, token], src_map -> [s-part, c]) are done on the host, so
the tensor engine runs only the productive bf16 matmuls.  exp(logits)
stays in SBUF between the A phase and the post-allreduce scale (no DRAM
round trip).
"""

import sys

for _p in ("/opt/trn_rl_repo", "/root/.axon_site/_ro/trn_rl_repo"):
    if _p not in sys.path:
        sys.path.insert(0, _p)

import numpy as np

import concourse.bass as bass
import concourse.mybir as mybir
from concourse import bacc, tile
from concourse.bass_utils import run_bass_kernel_spmd

f32 = mybir.dt.float32
bf16 = mybir.dt.bfloat16
P = 128

FULL_CFG = dict(B=16, T=128, S=512, C=512, V=50000, D=1024)
NCORES = 8
VTW = 512                       # vocab chunk width (one PSUM bank of f32)
LAG = 2                         # groups between A(g) and C(g)


def _ceil_div(a, b):
    return (a + b - 1) // b


def build_nc(cfg):
    B, T, S, C, V, D = (cfg[k] for k in ("B", "T", "S", "C", "V", "D"))
    BT = B * T
    VSH = V // NCORES           # vocab columns per core
    NT = BT // P                # token tiles of 128
    NK = D // P                 # contraction k-tiles
    NVT = _ceil_div(VSH, VTW)   # vocab chunks per core
    NS = S // P                 # copy-branch contraction k-tiles
    BSH = B // NCORES           # batches per core (copy branch)
    # 2-tile groups pipeline the allreduce.
    GROUP_SIZES = [2] * (NT // 2) if NT % 2 == 0 else [1] * NT
    NG = len(GROUP_SIZES)

    nc = bacc.Bacc(
        "TRN2", target_bir_lowering=False, debug=False, num_devices=NCORES
    )
    wT4 = nc.declare_dram_parameter("wT4", [NVT, P, NK, VTW], bf16, isOutput=False)
    b_sh = nc.declare_dram_parameter("b_shard", [1, VSH], bf16, isOutput=False)
    hT4 = nc.declare_dram_parameter("hT4", [NT, P, NK, P], bf16, isOutput=False)
    wcp = nc.declare_dram_parameter("w_copyT", [P, NK], bf16, isOutput=False)
    bcp = nc.declare_dram_parameter("b_copy", [1, 1], bf16, isOutput=False)
    attnT = nc.declare_dram_parameter("attnT", [P, NS, BSH * T], bf16, isOutput=False)
    srcT4 = nc.declare_dram_parameter("srcT4", [BSH, P, NS, C], bf16, isOutput=False)
    hcbT = nc.declare_dram_parameter("hcbT", [BSH, P, NK, P], bf16, isOutput=False)
    out_p = nc.declare_dram_parameter("out_prob", [BT, VSH], f32, isOutput=True)
    out_c = nc.declare_dram_parameter("copy_prob", [BSH * T, C], f32, isOutput=True)

    Exp = mybir.ActivationFunctionType.Exp
    Copy = mybir.ActivationFunctionType.Copy
    add = mybir.AluOpType.add
    mult = mybir.AluOpType.mult

    with tile.TileContext(nc, num_cores=NCORES) as tc:
        from contextlib import ExitStack

        with ExitStack() as stack:
            constp = stack.enter_context(tc.tile_pool(name="const", bufs=1))
            persist = stack.enter_context(tc.tile_pool(name="persist", bufs=1))
            hstp = stack.enter_context(tc.tile_pool(name="hT", bufs=3))
            sumsp = stack.enter_context(tc.tile_pool(name="sums", bufs=3))
            smallp = stack.enter_context(tc.tile_pool(name="small", bufs=8))
            lsgp = stack.enter_context(tc.tile_pool(name="lsg", bufs=4))
            psmm = stack.enter_context(
                tc.tile_pool(name="psum_mm", bufs=6, space="PSUM"))
            pssm = stack.enter_context(
                tc.tile_pool(name="psum_sm", bufs=1, space="PSUM"))
            dramp = stack.enter_context(
                tc.tile_pool(name="ccdram", bufs=2 * NG + 2, space="DRAM"))

            # ---- dummy allreduce: absorbs cross-core launch skew and CC
            #      warm-up while the tensor engine is anyway waiting on the
            #      initial W load, so the first real allreduce runs at
            #      steady-state latency.  Data is garbage and unused; the op
            #      only occupies the collective queue. ----
            warm_in = dramp.tile([P, 1], f32, tag="warm")
            warm_out = dramp.tile([P, 1], f32, tag="warm")
            nc.gpsimd.collective_compute(
                "AllReduce", mybir.AluOpType.add,
                replica_groups=[list(range(NCORES))],
                ins=[warm_in.opt()], outs=[warm_out.opt()],
            )

            # ---- constants ----
            ones1 = constp.tile([1, P], bf16)
            nc.gpsimd.memset(ones1[:, :], 1.0)
            wcT = constp.tile([P, NK], bf16)
            nc.sync.dma_start(wcT[:, :], wcp.ap())
            bcT = constp.tile([1, 1], bf16)
            nc.sync.dma_start(bcT[:, :], bcp.ap())
            bc_ps = pssm.tile([P, 1], f32, tag="pc", bufs=1)
            nc.tensor.matmul(bc_ps[:, :], ones1[0:1, :], bcT[0:1, :],
                             start=True, stop=True)
            bcNeg = constp.tile([P, 1], f32)
            nc.vector.tensor_scalar(bcNeg[:, :], bc_ps[:, :], -1.0, None, mult)

            # ---- W shard tiles (loads are issued later, after all small
            #      DMAs, so those don't queue behind 1 MB W transfers) ----
            def _wsz(vt):
                return min(VTW, VSH - vt * VTW)

            wt_t = []
            for vt in range(NVT):
                wsz = _wsz(vt)
                wtile = persist.tile([P, NK, wsz], bf16, name=f"wT{vt}")
                wt_t.append(wtile)

            # ---- bias broadcast [P, VSH] bf16 ----
            b_bc = persist.tile([P, VSH], bf16)
            with tc.tile_pool(name="bload", bufs=1) as blp:
                b_row = blp.tile([1, VSH], bf16)
                nc.sync.dma_start(b_row[:, :], b_sh.ap())
                for vt in range(NVT):
                    c0 = vt * VTW
                    wsz = _wsz(vt)
                    pm = psmm.tile([P, VTW], f32, tag="mm")
                    nc.tensor.matmul(
                        pm[:, :wsz], ones1[0:1, :], b_row[0:1, c0 : c0 + wsz],
                        start=True, stop=True,
                    )
                    nc.vector.tensor_copy(out=b_bc[:, c0 : c0 + wsz],
                                          in_=pm[:, :wsz])

            pcall = persist.tile([P, NT], f32)
            S_all = persist.tile([P, NT], f32)

            # ---- copy branch (batch-parallel; scoped pools, freed before
            #      the exp pool opens) ----
            with tc.tile_pool(name="cb", bufs=2) as cbp, \
                 tc.tile_pool(name="cbsrc", bufs=2) as srcp, \
                 tc.tile_pool(name="cbattn", bufs=1) as atp:
                at = atp.tile([P, NS, BSH * T], bf16)
                nc.sync.dma_start(at[:, :, :], attnT.ap())
                for i in range(BSH):
                    hcb = hstp.tile([P, NK, P], bf16, tag="hT")
                    nc.sync.dma_start(hcb[:, :, :], hcbT.ap()[i])
                    pps = pssm.tile([P, 1], f32, tag="pc", bufs=1)
                    for k in range(NK):
                        nc.tensor.matmul(
                            pps[:, :], hcb[:, k, :], wcT[:, k : k + 1],
                            start=(k == 0), stop=(k == NK - 1),
                        )
                    ycb = smallp.tile([P, 1], f32, tag="sc")
                    nc.scalar.activation(
                        ycb[:, :], pps[:, :], Exp, bias=bcNeg[:, :], scale=-1.0,
                    )
                    t1 = smallp.tile([P, 1], f32, tag="sc")
                    nc.vector.tensor_scalar(t1[:, :], ycb[:, :], 1.0, None, add)
                    pcb = smallp.tile([P, 1], f32, tag="sc")
                    nc.vector.reciprocal(pcb[:, :], t1[:, :])

                    st = srcp.tile([P, NS, C], bf16, tag="srcT")
                    nc.sync.dma_start(st[:, :, :], srcT4.ap()[i])
                    cps = psmm.tile([P, C], f32, tag="mm")
                    for k in range(NS):
                        nc.tensor.matmul(
                            cps[:, :], at[:, k, i * P : (i + 1) * P],
                            st[:, k, :],
                            start=(k == 0), stop=(k == NS - 1),
                        )
                    cstg = cbp.tile([P, C], f32, tag="cstg")
                    nc.vector.tensor_scalar(cstg[:, :], cps[:, :], pcb[:, :],
                                            None, mult)
                    nc.sync.dma_start(out_c.ap()[i * P : (i + 1) * P, :],
                                      cstg[:, :])

            # prefetch the first token tiles before the big W loads go out
            ht_pref = {}
            for tt in range(min(3, NT)):
                htp_ = hstp.tile([P, NK, P], bf16, tag="hT")
                nc.sync.dma_start(htp_[:, :, :], hT4.ap()[tt])
                ht_pref[tt] = htp_

            # ---- W loads: split per k-slice and issued in vt order, so the
            #      ring round-robin makes chunk vt0 land first and the first
            #      token tile's matmuls stream behind the load instead of
            #      waiting for the whole 12.8 MB ----
            for vt in range(NVT):
                wsz = _wsz(vt)
                for k in range(NK):
                    nc.sync.dma_start(
                        wt_t[vt][:, k, :], wT4.ap()[vt][:, k, :wsz]
                    )

            # exp staging + out staging (opened after copy pools close)
            expp = stack.enter_context(
                tc.tile_pool(name="exp", bufs=2 * (LAG + 1)))
            outstp = stack.enter_context(tc.tile_pool(name="outst", bufs=6))

            def phase_a(tt):
                if tt in ht_pref:
                    ht = ht_pref.pop(tt)
                else:
                    ht = hstp.tile([P, NK, P], bf16, tag="hT")
                    nc.sync.dma_start(ht[:, :, :], hT4.ap()[tt])
                pps = pssm.tile([P, 1], f32, tag="pc", bufs=1)
                for k in range(NK):
                    nc.tensor.matmul(
                        pps[:, :], ht[:, k, :], wcT[:, k : k + 1],
                        start=(k == 0), stop=(k == NK - 1),
                    )
                nc.scalar.activation(
                    pcall[:, tt : tt + 1], pps[:, :], Exp,
                    bias=bcNeg[:, :], scale=-1.0,
                )
                expt = expp.tile([P, VSH], bf16, tag="exp")
                sums_vt = sumsp.tile([P, NVT], f32, tag="sums")
                for vt in range(NVT):
                    c0 = vt * VTW
                    nsz = _wsz(vt)
                    pm = psmm.tile([P, VTW], f32, tag="mm")
                    for k in range(NK):
                        nc.tensor.matmul(
                            pm[:, :nsz], ht[:, k, :], wt_t[vt][:, k, :nsz],
                            start=(k == 0), stop=(k == NK - 1),
                        )
                    nc.vector.tensor_tensor(
                        pm[:, :nsz], pm[:, :nsz], b_bc[:, c0 : c0 + nsz], add
                    )
                    nc.scalar.activation(
                        expt[:, c0 : c0 + nsz], pm[:, :nsz], Exp,
                        accum_out=sums_vt[:, vt : vt + 1],
                    )
                return expt, sums_vt

            def phase_c(tt, expt):
                # pcall holds y = exp(-(h@w_copy + b_copy)); the out_prob
                # scale is (1 - p_copy)/S = y/((1+y)*S).
                y = pcall[:, tt : tt + 1]
                t1 = smallp.tile([P, 1], f32, tag="sc")
                nc.vector.tensor_scalar(t1[:, :], y, 1.0, None, add)
                t2 = smallp.tile([P, 1], f32, tag="sc")
                nc.vector.tensor_tensor(t2[:, :], t1[:, :],
                                        S_all[:, tt : tt + 1], mult)
                t3 = smallp.tile([P, 1], f32, tag="sc")
                nc.vector.reciprocal(t3[:, :], t2[:, :])
                rs = smallp.tile([P, 1], f32, tag="sc")
                nc.vector.tensor_tensor(rs[:, :], t3[:, :], y, mult)
                for j, c0 in enumerate(range(0, VSH, VTW)):
                    nsz = min(VTW, VSH - c0)
                    outst = outstp.tile([P, VTW], f32, tag="outst")
                    if j % 2 == 0:
                        nc.vector.tensor_scalar(
                            outst[:, :nsz], expt[:, c0 : c0 + nsz],
                            rs[:, :], None, mult,
                        )
                    else:
                        nc.scalar.activation(
                            outst[:, :nsz], expt[:, c0 : c0 + nsz], Copy,
                            scale=rs[:, :],
                        )
                    nc.sync.dma_start(
                        out_p.ap()[tt * P : (tt + 1) * P, c0 : c0 + nsz],
                        outst[:, :nsz],
                    )

            groups = []
            tt0 = 0
            for gsz in GROUP_SIZES:
                groups.append(list(range(tt0, tt0 + gsz)))
                tt0 += gsz
            assert tt0 == NT

            # A(g) then its allreduce; C(g) is emitted after A(g+LAG) so
            # the group-g allreduce has LAG groups of matmul time to
            # complete before anything waits on it.
            exp_store = {}
            for g, grp in enumerate(groups):
                lsg = lsgp.tile([P, len(grp)], f32, tag="lsg")
                for j, tt in enumerate(grp):
                    expt, sums_vt = phase_a(tt)
                    exp_store[tt] = expt
                    nc.vector.tensor_reduce(
                        lsg[:, j : j + 1], sums_vt[:, :NVT],
                        mybir.AxisListType.X, add,
                    )
                # cc_in / S_all-landing DMAs go on the gpsimd queue: the
                # landing DMA waits on the allreduce, and on the (in-order)
                # sync queue that wait would head-block the next group's
                # input loads and the out-prob stores.
                cc_in = dramp.tile([P, len(grp)], f32, tag="cc_in")
                cc_out = dramp.tile([P, len(grp)], f32, tag="cc_out")
                nc.gpsimd.dma_start(cc_in[:, :], lsg[:, :])
                nc.gpsimd.collective_compute(
                    "AllReduce", add,
                    replica_groups=[list(range(NCORES))],
                    ins=[cc_in.opt()], outs=[cc_out.opt()],
                )
                nc.gpsimd.dma_start(
                    S_all[:, grp[0] : grp[0] + len(grp)], cc_out[:, :]
                )
                if g >= LAG:
                    for tt in groups[g - LAG]:
                        phase_c(tt, exp_store.pop(tt))
            for g in range(max(0, NG - LAG), NG):
                for tt in groups[g]:
                    phase_c(tt, exp_store.pop(tt))

    nc.finalize()
    return nc


_CACHE = {}


def _get_nc(key, cfg):
    if key not in _CACHE:
        _CACHE[key] = build_nc(cfg)
    return _CACHE[key]


def make_in_maps(cfg, hidden, attn, src_map, W, b, w_copy, b_copy, pad_idx):
    B, T, S, C, V, D = (cfg[k] for k in ("B", "T", "S", "C", "V", "D"))
    BT = B * T
    VSH = V // NCORES
    NT = BT // P
    NK = D // P
    NVT = _ceil_div(VSH, VTW)
    NS = S // P
    BSH = B // NCORES
    hidden = np.asarray(hidden, dtype=np.float32)
    attn = np.asarray(attn, dtype=np.float32)
    src_map = np.asarray(src_map, dtype=np.float32)
    W = np.asarray(W, dtype=np.float32)
    b = np.asarray(b, dtype=np.float32)
    import ml_dtypes

    bF = ml_dtypes.bfloat16
    # hidden -> [NT, P(d%128), NK, P(token)] bf16
    HT = hidden.T.astype(bF)                                  # [D, BT]
    hT4 = np.ascontiguousarray(
        HT.reshape(NK, P, NT, P).transpose(2, 1, 0, 3)
    )
    w_copyT = np.ascontiguousarray(
        np.asarray(w_copy, dtype=np.float32).reshape(NK, P).T.astype(bF)
    )
    b_copy = np.asarray(b_copy, dtype=np.float32).reshape(1, 1).astype(bF)
    pad = int(np.asarray(pad_idx))

    in_maps = []
    for c in range(NCORES):
        lo, hi = c * VSH, (c + 1) * VSH
        WT = W[lo:hi].T.astype(bF)                            # [D, VSH]
        WTp = np.zeros((D, NVT * VTW), dtype=bF)
        WTp[:, :VSH] = WT
        wT4 = np.ascontiguousarray(
            WTp.reshape(NK, P, NVT, VTW).transpose(2, 1, 0, 3)
        )
        bsl = b[lo:hi].copy()
        if lo <= pad < hi:
            bsl[pad - lo] = -1e30
        ash = attn[c * BSH * T : (c + 1) * BSH * T]           # [BSH*T, S]
        attnT_a = np.ascontiguousarray(
            ash.T.reshape(NS, P, BSH * T).transpose(1, 0, 2).astype(bF)
        )
        ssh = src_map[c * BSH : (c + 1) * BSH]                # [BSH, S, C]
        srcT4_a = np.ascontiguousarray(
            ssh.reshape(BSH, NS, P, C).transpose(0, 2, 1, 3).astype(bF)
        )
        hcbT_a = np.ascontiguousarray(hT4[c * BSH : (c + 1) * BSH])
        in_maps.append(
            {
                "wT4": wT4,
                "b_shard": np.ascontiguousarray(bsl.astype(bF).reshape(1, VSH)),
                "hT4": hT4,
                "w_copyT": w_copyT,
                "b_copy": b_copy,
                "attnT": attnT_a,
                "srcT4": srcT4_a,
                "hcbT": hcbT_a,
            }
        )
    return in_maps


def assemble(cfg, results):
    out_prob = np.concatenate([r["out_prob"] for r in results], axis=1)
    copy_prob = np.concatenate([r["copy_prob"] for r in results], axis=0)
    return np.concatenate([out_prob, copy_prob], axis=1)


def run(cfg, inputs, trace=False):
    nc = _get_nc(tuple(sorted(cfg.items())), cfg)
    in_maps = make_in_maps(cfg, **inputs)
    res = run_bass_kernel_spmd(
        nc, in_maps, list(range(NCORES)), trace=trace
    )
    return assemble(cfg, res.results), res


def kernel(**inputs) -> np.ndarray:
    out, _ = run(FULL_CFG, inputs, trace=False)
    return out


# revision 37
# speedup vs baseline: 1.0415x; 1.0415x over previous
"""CopyGenerator kernel for 8x Trainium2 NeuronCores (Bass/Tile).

Computation (see reference):
    logits = hidden @ W.T + b            [BT, V]   (pad column masked to -inf)
    prob   = softmax(logits, axis=1)
    p_copy = sigmoid(hidden @ w_copy + b_copy)
    out    = concat([prob * (1 - p_copy),
                     einsum('bts,bsc', attn*p_copy, src_map)], axis=1)

Sharding: vocab dim of W/b/out_prob split 8 ways (tensor parallel).  Each
core computes exp(logits) for its vocab shard (bf16 matmul, tokens on
PSUM partitions), a per-token local sum-of-exp, an 8-core AllReduce of
the [BT] normalizer (tiny, pipelined in groups of 2 token tiles), and
scales+writes its out_prob columns.  The copy branch is data-parallel
over batch (2 batches per core).

All operand transposes (W -> [d-part, vocab], hidden -> [d-part, token],
attn -> [s-part, token], src_map -> [s-part, c]) are done on the host, so
the tensor engine runs only the productive bf16 matmuls.  exp(logits)
stays in SBUF between the A phase and the post-allreduce scale (no DRAM
round trip).
"""

import sys

for _p in ("/opt/trn_rl_repo", "/root/.axon_site/_ro/trn_rl_repo"):
    if _p not in sys.path:
        sys.path.insert(0, _p)

import numpy as np

import concourse.bass as bass
import concourse.mybir as mybir
from concourse import bacc, tile
from concourse.bass_utils import run_bass_kernel_spmd

f32 = mybir.dt.float32
bf16 = mybir.dt.bfloat16
P = 128

FULL_CFG = dict(B=16, T=128, S=512, C=512, V=50000, D=1024)
NCORES = 8
VTW = 512                       # vocab chunk width (one PSUM bank of f32)
LAG = 2                         # groups between A(g) and C(g)


def _ceil_div(a, b):
    return (a + b - 1) // b


def build_nc(cfg):
    B, T, S, C, V, D = (cfg[k] for k in ("B", "T", "S", "C", "V", "D"))
    BT = B * T
    VSH = V // NCORES           # vocab columns per core
    NT = BT // P                # token tiles of 128
    NK = D // P                 # contraction k-tiles
    NVT = _ceil_div(VSH, VTW)   # vocab chunks per core
    NS = S // P                 # copy-branch contraction k-tiles
    BSH = B // NCORES           # batches per core (copy branch)
    # 2-tile groups pipeline the allreduce; the last two groups are single
    # tiles so the tail (final allreduce + scale) is short.
    if NT >= 4 and NT % 2 == 0:
        GROUP_SIZES = [2] * (NT // 2 - 1) + [1, 1]
    else:
        GROUP_SIZES = [1] * NT
    NG = len(GROUP_SIZES)

    nc = bacc.Bacc(
        "TRN2", target_bir_lowering=False, debug=False, num_devices=NCORES
    )
    wT4 = nc.declare_dram_parameter("wT4", [NVT, P, NK, VTW], bf16, isOutput=False)
    b_sh = nc.declare_dram_parameter("b_shard", [1, VSH], bf16, isOutput=False)
    hT4 = nc.declare_dram_parameter("hT4", [NT, P, NK, P], bf16, isOutput=False)
    wcp = nc.declare_dram_parameter("w_copyT", [P, NK], bf16, isOutput=False)
    bcp = nc.declare_dram_parameter("b_copy", [1, 1], bf16, isOutput=False)
    attnT = nc.declare_dram_parameter("attnT", [P, NS, BSH * T], bf16, isOutput=False)
    srcT4 = nc.declare_dram_parameter("srcT4", [BSH, P, NS, C], bf16, isOutput=False)
    hcbT = nc.declare_dram_parameter("hcbT", [BSH, P, NK, P], bf16, isOutput=False)
    out_p = nc.declare_dram_parameter("out_prob", [BT, VSH], f32, isOutput=True)
    out_c = nc.declare_dram_parameter("copy_prob", [BSH * T, C], f32, isOutput=True)

    Exp = mybir.ActivationFunctionType.Exp
    Copy = mybir.ActivationFunctionType.Copy
    add = mybir.AluOpType.add
    mult = mybir.AluOpType.mult

    with tile.TileContext(nc, num_cores=NCORES) as tc:
        from contextlib import ExitStack

        with ExitStack() as stack:
            constp = stack.enter_context(tc.tile_pool(name="const", bufs=1))
            persist = stack.enter_context(tc.tile_pool(name="persist", bufs=1))
            hstp = stack.enter_context(tc.tile_pool(name="hT", bufs=3))
            sumsp = stack.enter_context(tc.tile_pool(name="sums", bufs=3))
            smallp = stack.enter_context(tc.tile_pool(name="small", bufs=8))
            lsgp = stack.enter_context(tc.tile_pool(name="lsg", bufs=4))
            psmm = stack.enter_context(
                tc.tile_pool(name="psum_mm", bufs=6, space="PSUM"))
            pssm = stack.enter_context(
                tc.tile_pool(name="psum_sm", bufs=1, space="PSUM"))
            dramp = stack.enter_context(
                tc.tile_pool(name="ccdram", bufs=2 * NG + 2, space="DRAM"))

            # ---- dummy allreduce: absorbs cross-core launch skew and CC
            #      warm-up while the tensor engine is anyway waiting on the
            #      initial W load, so the first real allreduce runs at
            #      steady-state latency.  Data is garbage and unused; the op
            #      only occupies the collective queue. ----
            warm_in = dramp.tile([P, 1], f32, tag="warm")
            warm_out = dramp.tile([P, 1], f32, tag="warm")
            nc.gpsimd.collective_compute(
                "AllReduce", mybir.AluOpType.add,
                replica_groups=[list(range(NCORES))],
                ins=[warm_in.opt()], outs=[warm_out.opt()],
            )

            # ---- constants ----
            ones1 = constp.tile([1, P], bf16)
            nc.gpsimd.memset(ones1[:, :], 1.0)
            wcT = constp.tile([P, NK], bf16)
            nc.sync.dma_start(wcT[:, :], wcp.ap())
            bcT = constp.tile([1, 1], bf16)
            nc.sync.dma_start(bcT[:, :], bcp.ap())
            bc_ps = pssm.tile([P, 1], f32, tag="pc", bufs=1)
            nc.tensor.matmul(bc_ps[:, :], ones1[0:1, :], bcT[0:1, :],
                             start=True, stop=True)
            bcNeg = constp.tile([P, 1], f32)
            nc.vector.tensor_scalar(bcNeg[:, :], bc_ps[:, :], -1.0, None, mult)

            # ---- W shard: host pre-transposed [d-part, vocab] bf16 ----
            def _wsz(vt):
                return min(VTW, VSH - vt * VTW)

            wt_t = []
            for vt in range(NVT):
                wsz = _wsz(vt)
                wtile = persist.tile([P, NK, wsz], bf16, name=f"wT{vt}")
                if wsz == VTW:
                    nc.sync.dma_start(wtile[:, :, :], wT4.ap()[vt])
                else:
                    nc.sync.dma_start(wtile[:, :, :], wT4.ap()[vt][:, :, :wsz])
                wt_t.append(wtile)

            # ---- bias broadcast [P, VSH] bf16 ----
            b_bc = persist.tile([P, VSH], bf16)
            with tc.tile_pool(name="bload", bufs=1) as blp:
                b_row = blp.tile([1, VSH], bf16)
                nc.sync.dma_start(b_row[:, :], b_sh.ap())
                for vt in range(NVT):
                    c0 = vt * VTW
                    wsz = _wsz(vt)
                    pm = psmm.tile([P, VTW], f32, tag="mm")
                    nc.tensor.matmul(
                        pm[:, :wsz], ones1[0:1, :], b_row[0:1, c0 : c0 + wsz],
                        start=True, stop=True,
                    )
                    nc.vector.tensor_copy(out=b_bc[:, c0 : c0 + wsz],
                                          in_=pm[:, :wsz])

            pcall = persist.tile([P, NT], f32)
            S_all = persist.tile([P, NT], f32)

            # ---- copy branch (batch-parallel; scoped pools, freed before
            #      the exp pool opens) ----
            with tc.tile_pool(name="cb", bufs=2) as cbp, \
                 tc.tile_pool(name="cbsrc", bufs=2) as srcp, \
                 tc.tile_pool(name="cbattn", bufs=1) as atp:
                at = atp.tile([P, NS, BSH * T], bf16)
                nc.sync.dma_start(at[:, :, :], attnT.ap())
                for i in range(BSH):
                    hcb = hstp.tile([P, NK, P], bf16, tag="hT")
                    nc.sync.dma_start(hcb[:, :, :], hcbT.ap()[i])
                    pps = pssm.tile([P, 1], f32, tag="pc", bufs=1)
                    for k in range(NK):
                        nc.tensor.matmul(
                            pps[:, :], hcb[:, k, :], wcT[:, k : k + 1],
                            start=(k == 0), stop=(k == NK - 1),
                        )
                    ycb = smallp.tile([P, 1], f32, tag="sc")
                    nc.scalar.activation(
                        ycb[:, :], pps[:, :], Exp, bias=bcNeg[:, :], scale=-1.0,
                    )
                    t1 = smallp.tile([P, 1], f32, tag="sc")
                    nc.vector.tensor_scalar(t1[:, :], ycb[:, :], 1.0, None, add)
                    pcb = smallp.tile([P, 1], f32, tag="sc")
                    nc.vector.reciprocal(pcb[:, :], t1[:, :])

                    st = srcp.tile([P, NS, C], bf16, tag="srcT")
                    nc.sync.dma_start(st[:, :, :], srcT4.ap()[i])
                    cps = psmm.tile([P, C], f32, tag="mm")
                    for k in range(NS):
                        nc.tensor.matmul(
                            cps[:, :], at[:, k, i * P : (i + 1) * P],
                            st[:, k, :],
                            start=(k == 0), stop=(k == NS - 1),
                        )
                    cstg = cbp.tile([P, C], f32, tag="cstg")
                    nc.vector.tensor_scalar(cstg[:, :], cps[:, :], pcb[:, :],
                                            None, mult)
                    nc.sync.dma_start(out_c.ap()[i * P : (i + 1) * P, :],
                                      cstg[:, :])

            # exp staging + out staging (opened after copy pools close)
            expp = stack.enter_context(
                tc.tile_pool(name="exp", bufs=2 * (LAG + 1)))
            outstp = stack.enter_context(tc.tile_pool(name="outst", bufs=6))

            def phase_a(tt):
                ht = hstp.tile([P, NK, P], bf16, tag="hT")
                nc.sync.dma_start(ht[:, :, :], hT4.ap()[tt])
                pps = pssm.tile([P, 1], f32, tag="pc", bufs=1)
                for k in range(NK):
                    nc.tensor.matmul(
                        pps[:, :], ht[:, k, :], wcT[:, k : k + 1],
                        start=(k == 0), stop=(k == NK - 1),
                    )
                nc.scalar.activation(
                    pcall[:, tt : tt + 1], pps[:, :], Exp,
                    bias=bcNeg[:, :], scale=-1.0,
                )
                expt = expp.tile([P, VSH], bf16, tag="exp")
                sums_vt = sumsp.tile([P, NVT], f32, tag="sums")
                for vt in range(NVT):
                    c0 = vt * VTW
                    nsz = _wsz(vt)
                    pm = psmm.tile([P, VTW], f32, tag="mm")
                    for k in range(NK):
                        nc.tensor.matmul(
                            pm[:, :nsz], ht[:, k, :], wt_t[vt][:, k, :nsz],
                            start=(k == 0), stop=(k == NK - 1),
                        )
                    nc.vector.tensor_tensor(
                        pm[:, :nsz], pm[:, :nsz], b_bc[:, c0 : c0 + nsz], add
                    )
                    nc.scalar.activation(
                        expt[:, c0 : c0 + nsz], pm[:, :nsz], Exp,
                        accum_out=sums_vt[:, vt : vt + 1],
                    )
                return expt, sums_vt

            def phase_c(tt, expt):
                # pcall holds y = exp(-(h@w_copy + b_copy)); the out_prob
                # scale is (1 - p_copy)/S = y/((1+y)*S).
                y = pcall[:, tt : tt + 1]
                t1 = smallp.tile([P, 1], f32, tag="sc")
                nc.vector.tensor_scalar(t1[:, :], y, 1.0, None, add)
                t2 = smallp.tile([P, 1], f32, tag="sc")
                nc.vector.tensor_tensor(t2[:, :], t1[:, :],
                                        S_all[:, tt : tt + 1], mult)
                t3 = smallp.tile([P, 1], f32, tag="sc")
                nc.vector.reciprocal(t3[:, :], t2[:, :])
                rs = smallp.tile([P, 1], f32, tag="sc")
                nc.vector.tensor_tensor(rs[:, :], t3[:, :], y, mult)
                for j, c0 in enumerate(range(0, VSH, VTW)):
                    nsz = min(VTW, VSH - c0)
                    outst = outstp.tile([P, VTW], f32, tag="outst")
                    if j % 2 == 0:
                        nc.vector.tensor_scalar(
                            outst[:, :nsz], expt[:, c0 : c0 + nsz],
                            rs[:, :], None, mult,
                        )
                    else:
                        nc.scalar.activation(
                            outst[:, :nsz], expt[:, c0 : c0 + nsz], Copy,
                            scale=rs[:, :],
                        )
                    nc.sync.dma_start(
                        out_p.ap()[tt * P : (tt + 1) * P, c0 : c0 + nsz],
                        outst[:, :nsz],
                    )

            groups = []
            tt0 = 0
            for gsz in GROUP_SIZES:
                groups.append(list(range(tt0, tt0 + gsz)))
                tt0 += gsz
            assert tt0 == NT

            # A(g) then its allreduce; C(g) is emitted after A(g+LAG) so
            # the group-g allreduce has LAG groups of matmul time to
            # complete before anything waits on it.
            exp_store = {}
            for g, grp in enumerate(groups):
                lsg = lsgp.tile([P, len(grp)], f32, tag="lsg")
                for j, tt in enumerate(grp):
                    expt, sums_vt = phase_a(tt)
                    exp_store[tt] = expt
                    nc.vector.tensor_reduce(
                        lsg[:, j : j + 1], sums_vt[:, :NVT],
                        mybir.AxisListType.X, add,
                    )
                # cc_in / S_all-landing DMAs go on the gpsimd queue: the
                # landing DMA waits on the allreduce, and on the (in-order)
                # sync queue that wait would head-block the next group's
                # input loads and the out-prob stores.
                cc_in = dramp.tile([P, len(grp)], f32, tag="cc_in")
                cc_out = dramp.tile([P, len(grp)], f32, tag="cc_out")
                nc.gpsimd.dma_start(cc_in[:, :], lsg[:, :])
                nc.gpsimd.collective_compute(
                    "AllReduce", add,
                    replica_groups=[list(range(NCORES))],
                    ins=[cc_in.opt()], outs=[cc_out.opt()],
                )
                nc.gpsimd.dma_start(
                    S_all[:, grp[0] : grp[0] + len(grp)], cc_out[:, :]
                )
                if g >= LAG:
                    for tt in groups[g - LAG]:
                        phase_c(tt, exp_store.pop(tt))
            for g in range(max(0, NG - LAG), NG):
                for tt in groups[g]:
                    phase_c(tt, exp_store.pop(tt))

    nc.finalize()
    return nc


_CACHE = {}


def _get_nc(key, cfg):
    if key not in _CACHE:
        _CACHE[key] = build_nc(cfg)
    return _CACHE[key]


def make_in_maps(cfg, hidden, attn, src_map, W, b, w_copy, b_copy, pad_idx):
    B, T, S, C, V, D = (cfg[k] for k in ("B", "T", "S", "C", "V", "D"))
    BT = B * T
    VSH = V // NCORES
    NT = BT // P
    NK = D // P
    NVT = _ceil_div(VSH, VTW)
    NS = S // P
    BSH = B // NCORES
    hidden = np.asarray(hidden, dtype=np.float32)
    attn = np.asarray(attn, dtype=np.float32)
    src_map = np.asarray(src_map, dtype=np.float32)
    W = np.asarray(W, dtype=np.float32)
    b = np.asarray(b, dtype=np.float32)
    import ml_dtypes

    bF = ml_dtypes.bfloat16
    # hidden -> [NT, P(d%128), NK, P(token)] bf16
    HT = hidden.T.astype(bF)                                  # [D, BT]
    hT4 = np.ascontiguousarray(
        HT.reshape(NK, P, NT, P).transpose(2, 1, 0, 3)
    )
    w_copyT = np.ascontiguousarray(
        np.asarray(w_copy, dtype=np.float32).reshape(NK, P).T.astype(bF)
    )
    b_copy = np.asarray(b_copy, dtype=np.float32).reshape(1, 1).astype(bF)
    pad = int(np.asarray(pad_idx))

    in_maps = []
    for c in range(NCORES):
        lo, hi = c * VSH, (c + 1) * VSH
        WT = W[lo:hi].T.astype(bF)                            # [D, VSH]
        WTp = np.zeros((D, NVT * VTW), dtype=bF)
        WTp[:, :VSH] = WT
        wT4 = np.ascontiguousarray(
            WTp.reshape(NK, P, NVT, VTW).transpose(2, 1, 0, 3)
        )
        bsl = b[lo:hi].copy()
        if lo <= pad < hi:
            bsl[pad - lo] = -1e30
        ash = attn[c * BSH * T : (c + 1) * BSH * T]           # [BSH*T, S]
        attnT_a = np.ascontiguousarray(
            ash.T.reshape(NS, P, BSH * T).transpose(1, 0, 2).astype(bF)
        )
        ssh = src_map[c * BSH : (c + 1) * BSH]                # [BSH, S, C]
        srcT4_a = np.ascontiguousarray(
            ssh.reshape(BSH, NS, P, C).transpose(0, 2, 1, 3).astype(bF)
        )
        hcbT_a = np.ascontiguousarray(hT4[c * BSH : (c + 1) * BSH])
        in_maps.append(
            {
                "wT4": wT4,
                "b_shard": np.ascontiguousarray(bsl.astype(bF).reshape(1, VSH)),
                "hT4": hT4,
                "w_copyT": w_copyT,
                "b_copy": b_copy,
                "attnT": attnT_a,
                "srcT4": srcT4_a,
                "hcbT": hcbT_a,
            }
        )
    return in_maps


def assemble(cfg, results):
    out_prob = np.concatenate([r["out_prob"] for r in results], axis=1)
    copy_prob = np.concatenate([r["copy_prob"] for r in results], axis=0)
    return np.concatenate([out_prob, copy_prob], axis=1)


def run(cfg, inputs, trace=False):
    nc = _get_nc(tuple(sorted(cfg.items())), cfg)
    in_maps = make_in_maps(cfg, **inputs)
    res = run_bass_kernel_spmd(
        nc, in_maps, list(range(NCORES)), trace=trace
    )
    return assemble(cfg, res.results), res


def kernel(**inputs) -> np.ndarray:
    out, _ = run(FULL_CFG, inputs, trace=False)
    return out
